# revision 30
# baseline (speedup 1.0000x reference)
"""CrossAttention Trainium2 kernel.

Problem (hardcoded): B=8, T=256, S=4096, E=512, KV=768, H=8, D=64.
Sharding: data-parallel over B — one batch per NeuronCore (8 cores).

Key optimization vs v0: the key_padding_mask drops ~half the keys and
softmax attention is permutation-invariant over keys, so the host gathers
only the kept context rows per batch and pads to S_pad (multiple of 128,
same for all cores). Padded rows have zero context => K=0 => scores=0 =>
exp=1, but their V' rows and denominator ones-column are zero, so they
contribute nothing. This exactly halves KV-proj / scores / exp / PV work.

Per-core dataflow (one batch, layouts staged host-side, bf16 unless noted):
  ctxT  [768, S_pad]        context[b][kept].T, zero-padded
  xT    [512, 256]          x[b].T
  onesp [128, n_sc*8]       per-(s,head) denominator ones-column (0 for pads)
  wqT   [512, 512]          Wq.T * D^-0.5 (scale folded)
  wkvT  [768, 1024]         Wkv.T
  woT   [512, 512]          Wo.T
  bo_r  [128, 4] f32        bo.reshape(4,128).T
Device, per S piece (512 cols at a time):
  KT    = wkvT[:, :512].T @ ctxT   -> [512c, S_pad]  (c-major)
  V'    = ctxT.T @ wkvT[:, 512:]   -> [S_pad, 8h*65] (64 vals + ones col)
  scoresT[s,t]/head: KT_h slices as lhsT, QT_h as rhs (K=64; head pair in
          PE row groups 0:64 / 64:128 runs concurrently)
  expsT = Exp(scoresT) (no max subtraction: |scores| small by construction)
  PV   += V'_h-as-lhsT @ expsT -> [65, 256]; row 64 = softmax denominator
Tail: reciprocal of denominators, K=1 matmul broadcast, OT = PV * recip,
  outT = woT.T @ OT + bo -> [512e, 256t] -> host transposes back.

DMA: few big strided transfers spread across the sync/gpsimd/vector/scalar
queues (issue cost ~630ns/call), ctx piece 0 split across two queues so
compute starts ASAP.
"""

import sys

sys.path.insert(0, "/opt/trn_rl_repo")

import numpy as np
import ml_dtypes
from contextlib import ExitStack

import concourse.bass as bass
import concourse.bacc as bacc
import concourse.tile as tile
from concourse import mybir
from concourse import bass_utils

BF16 = mybir.dt.bfloat16
F32 = mybir.dt.float32
NPBF16 = ml_dtypes.bfloat16

B, T, S, E, KV, H, D = 8, 256, 4096, 512, 768, 8, 64
NC_CORES = 8


def _pieces(s_pad):
    out = []
    p0 = 0
    while p0 < s_pad:
        w = min(512, s_pad - p0)
        out.append((p0, w))
        p0 += w
    if len(out) > 2 and out[-1][1] < 512:
        # move the short remainder piece to second position: piece 0 keeps a
        # full 512 of work to bridge the DMA ramp, and the last piece also
        # stays full-width so the softmax-norm chains hide under it
        out = out[0:1] + out[-1:] + out[1:-1]
    return out


def _build_program(s_pad, debug=False):
    n_sc = s_pad // 128
    pieces = _pieces(s_pad)
    nc = bacc.Bacc("TRN2", target_bir_lowering=False, debug=False)

    ctxT_d = nc.dram_tensor("ctxT", [KV, s_pad], BF16, kind="ExternalInput").ap()
    ctx8_d = nc.dram_tensor("ctx8", [KV, s_pad], mybir.dt.float8e4, kind="ExternalInput").ap()
    xT_d = nc.dram_tensor("xT", [E, T], BF16, kind="ExternalInput").ap()
    onesp_d = nc.dram_tensor("onesp", [128, n_sc * 8], BF16, kind="ExternalInput").ap()
    wqT_d = nc.dram_tensor("wqT", [E, 512], BF16, kind="ExternalInput").ap()
    wkv8_d = nc.dram_tensor("wkv8", [KV, 512], mybir.dt.float8e4, kind="ExternalInput").ap()
    wkvT_d = nc.dram_tensor("wkvT", [KV, 512], BF16, kind="ExternalInput").ap()
    woh_d = nc.dram_tensor("woh", [64, 8 * 512], BF16, kind="ExternalInput").ap()
    bo_d = nc.dram_tensor("bo_r", [128, 4], F32, kind="ExternalInput").ap()
    outT_d = nc.dram_tensor("outT", [4, 128, T], F32, kind="ExternalOutput").ap()
    fence_d = nc.dram_tensor("fence", [1, 16], BF16, kind="ExternalOutput").ap()
    fence8_d = nc.dram_tensor("fence8", [1, 8], mybir.dt.float8e4, kind="ExternalOutput").ap()
    if debug:
        dbg_qt = nc.dram_tensor("dbg_qt", [128, 4 * T], BF16, kind="ExternalOutput").ap()
        dbg_kt = nc.dram_tensor("dbg_kt", [128, 4 * s_pad], BF16, kind="ExternalOutput").ap()
        dbg_vp = nc.dram_tensor("dbg_vp", [128, n_sc * 8 * 65], BF16, kind="ExternalOutput").ap()
        dbg_den = nc.dram_tensor("dbg_den", [8, T], F32, kind="ExternalOutput").ap()
        dbg_pv = nc.dram_tensor("dbg_pv", [8, 65, T], F32, kind="ExternalOutput").ap()
        dbg_ot = nc.dram_tensor("dbg_ot", [4, 128, T], BF16, kind="ExternalOutput").ap()

    with tile.TileContext(nc) as tc, ExitStack() as ctx:
        const = ctx.enter_context(tc.tile_pool(name="const", bufs=1))
        work = ctx.enter_context(tc.tile_pool(name="work", bufs=2))
        p_pe = ctx.enter_context(tc.tile_pool(name="p_pe", bufs=3, space="PSUM"))
        p_pv = ctx.enter_context(tc.tile_pool(name="p_pv", bufs=2, space="PSUM"))

        # ---- static SBUF tensors -------------------------------------------
        ctx_t = const.tile([128, 6 * s_pad], BF16, tag="ctx")
        kt_t = const.tile([128, 4 * s_pad], BF16, tag="kt")
        vp_t = const.tile([128, n_sc * 8 * 65], BF16, tag="vp")
        qt_t = const.tile([128, 4 * T], BF16, tag="qt")
        oth_t = [const.tile([64, T], BF16, tag=f"oth{h}", name=f"oth{h}") for h in range(8)]
        wq_t = const.tile([128, 4 * 512], BF16, tag="wq")
        wkv_t = const.tile([128, 6 * 512], BF16, tag="wkv")
        wkv8_t = const.tile([128, 6 * 512], mybir.dt.float8e4, tag="wkv8")
        ctx8_t = const.tile([128, 6 * s_pad], mybir.dt.float8e4, tag="ctx8")
        woh_t = const.tile([64, 8 * 512], BF16, tag="woh")
        x_t = const.tile([128, 4 * T], BF16, tag="x")
        pvacc_t = [const.tile([65, T], F32, tag=f"pvacc{h}", name=f"pvacc{h}") for h in range(8)]
        onesf_t = const.tile([128, 64], F32, tag="onesf")
        bo_t = const.tile([128, 4], F32, tag="bo")
        osb_t = const.tile([128, 4 * T], F32, tag="osb")
        m01h_t = const.tile([128, n_sc * 8], BF16, tag="m01h")

        vp_v = vp_t[:].rearrange("p (sc h e) -> p sc h e", h=8, e=65)

        # ---- loads ----------------------------------------------------------
        nc.vector.memset(onesf_t[:], 1.0)
        ctx_sb = ctx_t[:].rearrange("p (c s) -> p c s", s=s_pad)
        ctx_dr = ctxT_d.rearrange("(c p) s -> p c s", p=128)
        ctx8_sb = ctx8_t[:].rearrange("p (c s) -> p c s", s=s_pad)
        wkv8_v = wkv8_t[:].rearrange("p (c m) -> p c m", m=512)
        p0w0 = pieces[0][1]
        # piece-0 critical path, finest-grained first: per-c chunks of
        # (wkv, ctx piece0) alternate sync/gpsimd so the c-ordered K-proj
        # accumulation can start as soon as chunk 0 lands.
        p00 = pieces[0][0]
        ctx8_dr = ctx8_d.rearrange("(c p) s -> p c s", p=128)
        # K-path critical set first: wkv8 + piece-0 ctx8 (fp8: tiny), with
        # fences so these complete before the queues start the bulk
        nc.sync.dma_start(
            wkv8_t[:].rearrange("p (c m) -> p c m", m=512),
            wkv8_d.rearrange("(c p) m -> p c m", p=128),
        )
        nc.gpsimd.dma_start(
            ctx8_sb[:, 0:3, p00 : p00 + p0w0], ctx8_dr[:, 0:3, p00 : p00 + p0w0]
        )
        nc.sync.dma_start(
            x_t[:].rearrange("p (c t) -> p c t", t=T),
            xT_d.rearrange("(c p) t -> p c t", p=128),
        )
        nc.gpsimd.dma_start(
            ctx8_sb[:, 3:6, p00 : p00 + p0w0], ctx8_dr[:, 3:6, p00 : p00 + p0w0]
        )
        nc.sync.dma_start(fence8_d[0:1, 0:8], wkv8_t[0:1, 0:8])
        nc.gpsimd.dma_start(
            fence8_d[0:1, 0:8], ctx8_t[0:1, p00 : p00 + 8]
        )
        nc.gpsimd.dma_start(
            wq_t[:].rearrange("p (c m) -> p c m", m=512),
            wqT_d.rearrange("(c p) m -> p c m", p=128),
        )
        # rest of ctx8 in one transfer (everything after piece 0)
        nc.sync.dma_start(
            ctx8_sb[:, :, p00 + p0w0 : s_pad], ctx8_dr[:, :, p00 + p0w0 : s_pad]
        )
        # bf16 ctx (V-path): piece 0 split, then the rest
        nc.gpsimd.dma_start(
            ctx_sb[:, 0:3, p00 : p00 + p0w0], ctx_dr[:, 0:3, p00 : p00 + p0w0]
        )
        nc.gpsimd.dma_start(
            ctx_sb[:, 3:6, p00 : p00 + p0w0], ctx_dr[:, 3:6, p00 : p00 + p0w0]
        )
        nc.gpsimd.dma_start(
            wkv_t[:].rearrange("p (c m) -> p c m", m=512),
            wkvT_d.rearrange("(c p) m -> p c m", p=128),
        )
        nc.scalar.dma_start(m01h_t[:], onesp_d)
        if len(pieces) > 1:
            p1, w1 = pieces[1]
            for c in range(6):
                eng = nc.sync if c % 2 == 0 else nc.gpsimd
                eng.dma_start(
                    ctx_sb[:, c : c + 1, p1 : p1 + w1],
                    ctx_dr[:, c : c + 1, p1 : p1 + w1],
                )
        for i, (p0, w) in enumerate(pieces[2:]):
            eng = nc.sync if i % 2 == 0 else nc.gpsimd
            eng.dma_start(ctx_sb[:, :, p0 : p0 + w], ctx_dr[:, :, p0 : p0 + w])
        nc.scalar.dma_start(woh_t[:], woh_d)
        nc.scalar.dma_start(bo_t[:], bo_d)

        def emit_norm(kc):
            # DMA-free normalization: a K=1 fp32 matmul broadcasts the raw
            # denominator row (pvacc partition 64) over 64 psum partitions,
            # the fast reciprocal runs on the broadcast at partitions 0:64,
            # and the DVE multiply normalizes in place. No queue latency.
            for hh in (2 * kc, 2 * kc + 1):
                bcd = p_pv.tile([64, T], F32, tag="pv", name="pv")
                nc.tensor.matmul(
                    bcd[:],
                    lhsT=onesf_t[64:65, :],
                    rhs=pvacc_t[hh][64:65, :],
                    start=True,
                    stop=True,
                )
                recf = work.tile([64, T], F32, tag="recf", bufs=4, name="recf")
                nc.vector.reciprocal_approx_fast(recf[:], bcd[:])
                nc.vector.tensor_mul(oth_t[hh][:], pvacc_t[hh][0:64, :], recf[:])

        # ---- PE warmup: dummy matmuls on a memset tile keep the PE busy
        # through the initial DMA wait so HAM un-throttles (K=8/8) before the
        # first real matmul; otherwise the first ~7us of real work runs at
        # half clock. Results go to a scratch psum tile nothing reads.
        warm_t = const.tile([128, 512], BF16, tag="warm")
        nc.vector.memset(warm_t[:], 0.0)
        for wi in range(26):
            wps = p_pv.tile([64, T], F32, tag="pv", name="pv")
            nc.tensor.matmul(
                wps[:],
                lhsT=warm_t[:, 0:64],
                rhs=warm_t[:, 256 : 256 + T],
                start=True,
                stop=True,
            )

        # ---- main loop over S pieces (Q-proj slotted inside piece 0 so the
        # PE starts on K-proj as soon as the first wkv/ctx chunks land) ------
        for pi, (p0, w) in enumerate(pieces):
            scs = [p0 // 128 + j for j in range(w // 128)]
            # K projection: c-outer so each (wkv_c, ctx_c) chunk is consumed
            # as it arrives (keeps the PE from idling >HAM window at startup);
            # both kc-pair psum tiles stay live through the c loop.
            ps_h = [
                p_pe.tile([128, 1024], F32, tag="pe", name="pe") for _ in range(2)
            ]
            for g in range(3):
                for half in range(2):
                    for kci in range(2):
                        kc = 2 * half + kci
                        nc.tensor.matmul(
                            ps_h[half][:, kci * w : (kci + 1) * w],
                            lhsT=wkv8_v[:, 2 * g : 2 * g + 2, kc * 128 : (kc + 1) * 128],
                            rhs=ctx8_sb[:, 2 * g : 2 * g + 2, p0 : p0 + w],
                            start=(g == 0),
                            stop=(g == 2),
                            perf_mode=mybir.MatmulPerfMode.DoubleRow,
                        )
            kt_v = kt_t[:].rearrange("p (kc s) -> p kc s", s=s_pad)
            for half in range(2):
                nc.vector.tensor_copy(
                    kt_v[:, 2 * half : 2 * half + 2, p0 : p0 + w],
                    ps_h[half][:, 0 : 2 * w].rearrange("p (kc s) -> p kc s", s=w),
                )
            if pi == 0:
                # Q projection: x/wq arrive on the scalar queue while the
                # K-proj trickle runs; this also plugs a PE gap before V-proj
                ps_q = p_pe.tile([128, 1024], F32, tag="pe", name="pe")
                for qc in range(4):
                    for ec in range(4):
                        nc.tensor.matmul(
                            ps_q[:, qc * T : qc * T + T],
                            lhsT=wq_t[:, ec * 512 + qc * 128 : ec * 512 + (qc + 1) * 128],
                            rhs=x_t[:, ec * T : (ec + 1) * T],
                            start=(ec == 0),
                            stop=(ec == 3),
                        )
                nc.vector.tensor_copy(qt_t[:], ps_q[:])
            # scores + exp per head pair (before V-proj: exp pipeline starts
            # sooner and V-proj matmuls keep the PE dense during the exp tail)
            etiles = []
            for kc in range(4):
                pe0 = p_pe.tile([128, 1024], F32, tag="pe", name="pe")
                pe1 = p_pe.tile([128, 1024], F32, tag="pe", name="pe")
                for j, sc in enumerate(scs):
                    nc.tensor.matmul(
                        pe0[:, j * T : (j + 1) * T],
                        lhsT=kt_t[0:64, kc * s_pad + sc * 128 : kc * s_pad + (sc + 1) * 128],
                        rhs=qt_t[0:64, kc * T : (kc + 1) * T],
                        start=True,
                        stop=True,
                        tile_position=(0, 0),
                    )
                    nc.tensor.matmul(
                        pe1[:, j * T : (j + 1) * T],
                        lhsT=kt_t[64:128, kc * s_pad + sc * 128 : kc * s_pad + (sc + 1) * 128],
                        rhs=qt_t[64:128, kc * T : (kc + 1) * T],
                        start=True,
                        stop=True,
                        tile_position=(64, 0),
                    )
                e0 = work.tile([128, 1024], BF16, tag="exp", bufs=8, name="exp")
                nc.scalar.activation(
                    e0[:, 0 : w * 2], pe0[:, 0 : w * 2], mybir.ActivationFunctionType.Exp
                )
                e1 = work.tile([128, 1024], BF16, tag="exp", bufs=8, name="exp")
                nc.scalar.activation(
                    e1[:, 0 : w * 2], pe1[:, 0 : w * 2], mybir.ActivationFunctionType.Exp
                )
                etiles.append((e0, e1))
            # V' projection: sc pairs share one psum tile
            for g in range(0, len(scs), 2):
                pair = scs[g : g + 2]
                ps = p_pe.tile([128, 1024], F32, tag="pe", name="pe")
                for j, sc in enumerate(pair):
                    for c in range(6):
                        nc.tensor.matmul(
                            ps[:, j * 512 : (j + 1) * 512],
                            lhsT=ctx_t[:, c * s_pad + sc * 128 : c * s_pad + (sc + 1) * 128],
                            rhs=wkv_t[:, c * 512 : (c + 1) * 512],
                            start=(c == 0),
                            stop=(c == 5),
                        )
                nc.vector.tensor_copy(
                    vp_v[:, pair[0] : pair[0] + len(pair), :, 0:64],
                    ps[:, 0 : len(pair) * 512].rearrange(
                        "p (sc h d) -> p sc h d", h=8, d=64
                    ),
                )
            nc.vector.tensor_copy(
                vp_v[:, scs[0] : scs[0] + len(scs), :, 64:65],
                m01h_t[:, scs[0] * 8 : (scs[0] + len(scs)) * 8].rearrange(
                    "p (sc h o) -> p sc h o", h=8, o=1
                ),
            )
            # PV accumulation per head pair
            for kc in range(4):
                e0, e1 = etiles[kc]
                pv0 = p_pv.tile([65, T], F32, tag="pv", name="pv")
                pv1 = p_pv.tile([65, T], F32, tag="pv", name="pv")
                for j, sc in enumerate(scs):
                    o0 = (sc * 8 + 2 * kc) * 65
                    o1 = (sc * 8 + 2 * kc + 1) * 65
                    nc.tensor.matmul(
                        pv0[:],
                        lhsT=vp_t[:, o0 : o0 + 65],
                        rhs=e0[:, j * T : (j + 1) * T],
                        start=(j == 0),
                        stop=(j == len(scs) - 1),
                    )
                    nc.tensor.matmul(
                        pv1[:],
                        lhsT=vp_t[:, o1 : o1 + 65],
                        rhs=e1[:, j * T : (j + 1) * T],
                        start=(j == 0),
                        stop=(j == len(scs) - 1),
                    )
                if pi == 0:
                    nc.vector.tensor_copy(pvacc_t[2 * kc][:], pv0[:])
                    nc.vector.tensor_copy(pvacc_t[2 * kc + 1][:], pv1[:])
                else:
                    nc.vector.tensor_add(pvacc_t[2 * kc][:], pvacc_t[2 * kc][:], pv0[:])
                    nc.vector.tensor_add(
                        pvacc_t[2 * kc + 1][:], pvacc_t[2 * kc + 1][:], pv1[:]
                    )
                if pi == len(pieces) - 1:
                    if kc >= 1:
                        emit_norm(kc - 1)
                    if kc == 3:
                        emit_norm(2)
                        emit_norm(3)

        # ---- out projection: per-head K=64 contraction against the
        # head-major Wo layout; no partition shifts so each head's OT feeds
        # matmuls straight from the DVE mul. 4 accumulators in 4 banks.
        ps_outA = p_pe.tile([128, 1024], F32, tag="pe", name="pe")
        ps_outB = p_pe.tile([128, 1024], F32, tag="pe", name="pe")
        out_slot = [(ps_outA, 0), (ps_outA, 512), (ps_outB, 0), (ps_outB, 512)]
        for h in range(8):
            for eo in range(4):
                pt, off = out_slot[eo]
                nc.tensor.matmul(
                    pt[:, off : off + T],
                    lhsT=woh_t[:, h * 512 + eo * 128 : h * 512 + (eo + 1) * 128],
                    rhs=oth_t[h][:],
                    start=(h == 0),
                    stop=(h == 7),
                )
        for eo in range(4):
            pt, off = out_slot[eo]
            nc.vector.tensor_scalar_add(
                osb_t[:, eo * T : (eo + 1) * T],
                pt[:, off : off + T],
                bo_t[:, eo : eo + 1],
            )
            eng = nc.sync if eo % 2 == 0 else nc.gpsimd
            eng.dma_start(outT_d[eo], osb_t[:, eo * T : (eo + 1) * T])
        if debug:
            nc.gpsimd.dma_start(dbg_qt, qt_t[:])
            nc.gpsimd.dma_start(dbg_kt, kt_t[:])
            nc.gpsimd.dma_start(dbg_vp, vp_t[:])
            pass
            for h in range(8):
                nc.gpsimd.dma_start(dbg_pv[h], pvacc_t[h][:])
            for cc in range(4):
                nc.gpsimd.dma_start(dbg_ot[cc][0:64], oth_t[2 * cc][:])
                nc.gpsimd.dma_start(dbg_ot[cc][64:128], oth_t[2 * cc + 1][:])

    nc.compile()
    return nc


_NC_CACHE = {}


def _get_nc(s_pad, debug=False):
    key = (s_pad, debug)
    if key not in _NC_CACHE:
        _NC_CACHE[key] = _build_program(s_pad, debug)
    return _NC_CACHE[key]


def _prep_in_maps(x, context, key_padding_mask, Wq, Wkv, Wo, bo):
    keep = ~np.asarray(key_padding_mask)
    kept = keep.sum(axis=1)
    s_pad = max(128, -(-int(kept.max()) // 128) * 128)
    n_sc = s_pad // 128

    wqT = (np.ascontiguousarray(Wq.T) * np.float32(D**-0.5 / 32.0)).astype(NPBF16)
    wkv8 = np.ascontiguousarray(Wkv[:512].T * np.float32(32.0)).astype(
        ml_dtypes.float8_e4m3
    )
    wkvT = np.ascontiguousarray(Wkv[512:].T).astype(NPBF16)
    woh = np.ascontiguousarray(
        Wo.T.reshape(8, 64, 512).transpose(1, 0, 2).reshape(64, 8 * 512)
    ).astype(NPBF16)
    bo_r = np.ascontiguousarray(bo.reshape(4, 128).T).astype(np.float32)
    in_maps = []
    for b in range(B):
        k = int(kept[b])
        ctxT = np.zeros((KV, s_pad), dtype=NPBF16)
        ctxT[:, :k] = context[b][keep[b]].T.astype(NPBF16)
        ctx8 = np.zeros((KV, s_pad), dtype=ml_dtypes.float8_e4m3)
        ctx8[:, :k] = context[b][keep[b]].T.astype(ml_dtypes.float8_e4m3)
        xT = np.ascontiguousarray(x[b].T).astype(NPBF16)
        # ones-column pattern: onesp[p, sc*8 + h] = 1 if sc*128+p < k
        live = (np.arange(s_pad) < k).astype(NPBF16).reshape(n_sc, 128).T
        onesp = np.ascontiguousarray(np.repeat(live, 8, axis=1))
        in_maps.append(
            dict(ctxT=ctxT, ctx8=ctx8, xT=xT, onesp=onesp, wqT=wqT, wkv8=wkv8, wkvT=wkvT, woh=woh, bo_r=bo_r)
        )
    return s_pad, in_maps


def _run(inputs, trace=False, debug=False, **kw):
    s_pad, in_maps = _prep_in_maps(**inputs)
    nc = _get_nc(s_pad, debug)
    res = bass_utils.run_bass_kernel_spmd(
        nc, in_maps, core_ids=list(range(NC_CORES)), trace=trace, **kw
    )
    out = np.stack(
        [res.results[b]["outT"].reshape(E, T).T for b in range(B)]
    ).astype(np.float32)
    return out, res


def kernel(**inputs):
    out, _ = _run(inputs, trace=False)
    return out


if __name__ == "__main__":
    rng = np.random.default_rng(0)
    ins = dict(
        x=rng.standard_normal((B, T, E), dtype=np.float32),
        context=rng.standard_normal((B, S, KV), dtype=np.float32),
        key_padding_mask=rng.integers(0, 2, (B, S)).astype(bool),
        Wq=(rng.standard_normal((512, E), dtype=np.float32) * 0.02),
        Wkv=(rng.standard_normal((1024, KV), dtype=np.float32) * 0.02),
        Wo=(rng.standard_normal((E, 512), dtype=np.float32) * 0.02),
        bo=np.zeros(E, dtype=np.float32),
    )
    out = kernel(**ins)
    print("out", out.shape, out.dtype, np.abs(out).mean())


# revision 31
# speedup vs baseline: 1.1109x; 1.1109x over previous
"""CrossAttention Trainium2 kernel.

Problem (hardcoded): B=8, T=256, S=4096, E=512, KV=768, H=8, D=64.
Sharding: data-parallel over B — one batch per NeuronCore (8 cores).

Key optimization vs v0: the key_padding_mask drops ~half the keys and
softmax attention is permutation-invariant over keys, so the host gathers
only the kept context rows per batch and pads to S_pad (multiple of 128,
same for all cores). Padded rows have zero context => K=0 => scores=0 =>
exp=1, but their V' rows and denominator ones-column are zero, so they
contribute nothing. This exactly halves KV-proj / scores / exp / PV work.

Per-core dataflow (one batch, layouts staged host-side, bf16 unless noted):
  ctxT  [768, S_pad]        context[b][kept].T, zero-padded
  xT    [512, 256]          x[b].T
  onesp [128, n_sc*8]       per-(s,head) denominator ones-column (0 for pads)
  wqT   [512, 512]          Wq.T * D^-0.5 (scale folded)
  wkvT  [768, 1024]         Wkv.T
  woT   [512, 512]          Wo.T
  bo_r  [128, 4] f32        bo.reshape(4,128).T
Device, per S piece (512 cols at a time):
  KT    = wkvT[:, :512].T @ ctxT   -> [512c, S_pad]  (c-major)
  V'    = ctxT.T @ wkvT[:, 512:]   -> [S_pad, 8h*65] (64 vals + ones col)
  scoresT[s,t]/head: KT_h slices as lhsT, QT_h as rhs (K=64; head pair in
          PE row groups 0:64 / 64:128 runs concurrently)
  expsT = Exp(scoresT) (no max subtraction: |scores| small by construction)
  PV   += V'_h-as-lhsT @ expsT -> [65, 256]; row 64 = softmax denominator
Tail: reciprocal of denominators, K=1 matmul broadcast, OT = PV * recip,
  outT = woT.T @ OT + bo -> [512e, 256t] -> host transposes back.

DMA: few big strided transfers spread across the sync/gpsimd/vector/scalar
queues (issue cost ~630ns/call), ctx piece 0 split across two queues so
compute starts ASAP.
"""

import sys

sys.path.insert(0, "/opt/trn_rl_repo")

import numpy as np
import ml_dtypes
from contextlib import ExitStack

import concourse.bass as bass
import concourse.bacc as bacc
import concourse.tile as tile
from concourse import mybir
from concourse import bass_utils

BF16 = mybir.dt.bfloat16
F32 = mybir.dt.float32
NPBF16 = ml_dtypes.bfloat16

B, T, S, E, KV, H, D = 8, 256, 4096, 512, 768, 8, 64
NC_CORES = 8


def _pieces(s_pad):
    out = []
    p0 = 0
    while p0 < s_pad:
        w = min(512, s_pad - p0)
        out.append((p0, w))
        p0 += w
    if len(out) > 2 and out[-1][1] < 512:
        # move the short remainder piece to second position: piece 0 keeps a
        # full 512 of work to bridge the DMA ramp, and the last piece also
        # stays full-width so the softmax-norm chains hide under it
        out = out[0:1] + out[-1:] + out[1:-1]
    return out


def _build_program(s_pad, debug=False):
    n_sc = s_pad // 128
    pieces = _pieces(s_pad)
    nc = bacc.Bacc("TRN2", target_bir_lowering=False, debug=False)

    ctxT_d = nc.dram_tensor("ctxT", [KV, s_pad], BF16, kind="ExternalInput").ap()
    ctx8_d = nc.dram_tensor("ctx8", [KV, s_pad], mybir.dt.float8e4, kind="ExternalInput").ap()
    xT_d = nc.dram_tensor("xT", [E, T], BF16, kind="ExternalInput").ap()
    onesp_d = nc.dram_tensor("onesp", [128, n_sc * 8], BF16, kind="ExternalInput").ap()
    wqT_d = nc.dram_tensor("wqT", [E, 512], BF16, kind="ExternalInput").ap()
    wkv8_d = nc.dram_tensor("wkv8", [KV, 512], mybir.dt.float8e4, kind="ExternalInput").ap()
    wkvT_d = nc.dram_tensor("wkvT", [KV, 512], BF16, kind="ExternalInput").ap()
    woh_d = nc.dram_tensor("woh", [64, 8 * 512], BF16, kind="ExternalInput").ap()
    bo_d = nc.dram_tensor("bo_r", [128, 4], F32, kind="ExternalInput").ap()
    outT_d = nc.dram_tensor("outT", [4, 128, T], F32, kind="ExternalOutput").ap()
    fence_d = nc.dram_tensor("fence", [1, 16], BF16, kind="ExternalOutput").ap()
    fence8_d = nc.dram_tensor("fence8", [1, 8], mybir.dt.float8e4, kind="ExternalOutput").ap()
    if debug:
        dbg_qt = nc.dram_tensor("dbg_qt", [128, 4 * T], BF16, kind="ExternalOutput").ap()
        dbg_kt = nc.dram_tensor("dbg_kt", [128, 4 * s_pad], BF16, kind="ExternalOutput").ap()
        dbg_vp = nc.dram_tensor("dbg_vp", [128, n_sc * 8 * 65], BF16, kind="ExternalOutput").ap()
        dbg_den = nc.dram_tensor("dbg_den", [8, T], F32, kind="ExternalOutput").ap()
        dbg_pv = nc.dram_tensor("dbg_pv", [8, 65, T], F32, kind="ExternalOutput").ap()
        dbg_ot = nc.dram_tensor("dbg_ot", [4, 128, T], BF16, kind="ExternalOutput").ap()

    with tile.TileContext(nc) as tc, ExitStack() as ctx:
        const = ctx.enter_context(tc.tile_pool(name="const", bufs=1))
        work = ctx.enter_context(tc.tile_pool(name="work", bufs=2))
        p_pe = ctx.enter_context(tc.tile_pool(name="p_pe", bufs=3, space="PSUM"))
        p_pv = ctx.enter_context(tc.tile_pool(name="p_pv", bufs=2, space="PSUM"))

        # ---- static SBUF tensors -------------------------------------------
        ctx_t = const.tile([128, 6 * s_pad], BF16, tag="ctx")
        kt_t = const.tile([128, 4 * s_pad], BF16, tag="kt")
        vp_t = const.tile([128, n_sc * 8 * 65], BF16, tag="vp")
        qt_t = const.tile([128, 4 * T], BF16, tag="qt")
        oth_t = [const.tile([64, T], BF16, tag=f"oth{h}", name=f"oth{h}") for h in range(8)]
        wq_t = const.tile([128, 4 * 512], BF16, tag="wq")
        wkv_t = const.tile([128, 6 * 512], BF16, tag="wkv")
        wkv8_t = const.tile([128, 6 * 512], mybir.dt.float8e4, tag="wkv8")
        ctx8_t = const.tile([128, 6 * s_pad], mybir.dt.float8e4, tag="ctx8")
        woh_t = const.tile([64, 8 * 512], BF16, tag="woh")
        x_t = const.tile([128, 4 * T], BF16, tag="x")
        pvacc_t = [const.tile([65, T], F32, tag=f"pvacc{h}", name=f"pvacc{h}") for h in range(8)]
        onesf_t = const.tile([128, 64], F32, tag="onesf")
        bo_t = const.tile([128, 4], F32, tag="bo")
        osb_t = const.tile([128, 4 * T], F32, tag="osb")
        m01h_t = const.tile([128, n_sc * 8], BF16, tag="m01h")

        vp_v = vp_t[:].rearrange("p (sc h e) -> p sc h e", h=8, e=65)

        # ---- loads ----------------------------------------------------------
        nc.vector.memset(onesf_t[:], 1.0)
        ctx_sb = ctx_t[:].rearrange("p (c s) -> p c s", s=s_pad)
        ctx_dr = ctxT_d.rearrange("(c p) s -> p c s", p=128)
        ctx8_sb = ctx8_t[:].rearrange("p (c s) -> p c s", s=s_pad)
        wkv8_v = wkv8_t[:].rearrange("p (c m) -> p c m", m=512)
        p0w0 = pieces[0][1]
        # piece-0 critical path, finest-grained first: per-c chunks of
        # (wkv, ctx piece0) alternate sync/gpsimd so the c-ordered K-proj
        # accumulation can start as soon as chunk 0 lands.
        p00 = pieces[0][0]
        ctx8_dr = ctx8_d.rearrange("(c p) s -> p c s", p=128)
        # K-path critical set first: wkv8 + piece-0 ctx8 (fp8: tiny), with
        # fences so these complete before the queues start the bulk
        nc.sync.dma_start(
            wkv8_t[:].rearrange("p (c m) -> p c m", m=512),
            wkv8_d.rearrange("(c p) m -> p c m", p=128),
        )
        nc.gpsimd.dma_start(
            ctx8_sb[:, 0:3, p00 : p00 + p0w0], ctx8_dr[:, 0:3, p00 : p00 + p0w0]
        )
        nc.sync.dma_start(
            x_t[:].rearrange("p (c t) -> p c t", t=T),
            xT_d.rearrange("(c p) t -> p c t", p=128),
        )
        nc.gpsimd.dma_start(
            ctx8_sb[:, 3:6, p00 : p00 + p0w0], ctx8_dr[:, 3:6, p00 : p00 + p0w0]
        )
        nc.sync.dma_start(fence8_d[0:1, 0:8], wkv8_t[0:1, 0:8])
        nc.gpsimd.dma_start(
            fence8_d[0:1, 0:8], ctx8_t[0:1, p00 : p00 + 8]
        )
        nc.gpsimd.dma_start(
            wq_t[:].rearrange("p (c m) -> p c m", m=512),
            wqT_d.rearrange("(c p) m -> p c m", p=128),
        )
        # rest of ctx8 in one transfer (everything after piece 0)
        nc.sync.dma_start(
            ctx8_sb[:, :, p00 + p0w0 : s_pad], ctx8_dr[:, :, p00 + p0w0 : s_pad]
        )
        # bf16 ctx (V-path): piece 0 split, then the rest
        nc.gpsimd.dma_start(
            ctx_sb[:, 0:3, p00 : p00 + p0w0], ctx_dr[:, 0:3, p00 : p00 + p0w0]
        )
        nc.gpsimd.dma_start(
            ctx_sb[:, 3:6, p00 : p00 + p0w0], ctx_dr[:, 3:6, p00 : p00 + p0w0]
        )
        nc.gpsimd.dma_start(
            wkv_t[:].rearrange("p (c m) -> p c m", m=512),
            wkvT_d.rearrange("(c p) m -> p c m", p=128),
        )
        nc.scalar.dma_start(m01h_t[:], onesp_d)
        if len(pieces) > 1:
            p1, w1 = pieces[1]
            for c in range(6):
                eng = nc.sync if c % 2 == 0 else nc.gpsimd
                eng.dma_start(
                    ctx_sb[:, c : c + 1, p1 : p1 + w1],
                    ctx_dr[:, c : c + 1, p1 : p1 + w1],
                )
        for i, (p0, w) in enumerate(pieces[2:]):
            eng = nc.sync if i % 2 == 0 else nc.gpsimd
            eng.dma_start(ctx_sb[:, :, p0 : p0 + w], ctx_dr[:, :, p0 : p0 + w])
        nc.scalar.dma_start(woh_t[:], woh_d)
        nc.scalar.dma_start(bo_t[:], bo_d)

        def emit_norm(kc):
            # DMA-free normalization: a K=1 fp32 matmul broadcasts the raw
            # denominator row (pvacc partition 64) over 64 psum partitions,
            # the fast reciprocal runs on the broadcast at partitions 0:64,
            # and the DVE multiply normalizes in place. No queue latency.
            for hh in (2 * kc, 2 * kc + 1):
                bcd = p_pv.tile([64, T], F32, tag="pv", name="pv")
                nc.tensor.matmul(
                    bcd[:],
                    lhsT=onesf_t[64:65, :],
                    rhs=pvacc_t[hh][64:65, :],
                    start=True,
                    stop=True,
                )
                recf = work.tile([64, T], F32, tag="recf", bufs=4, name="recf")
                nc.vector.reciprocal_approx_fast(recf[:], bcd[:])
                nc.vector.tensor_mul(oth_t[hh][:], pvacc_t[hh][0:64, :], recf[:])

        # ---- PE warmup: dummy matmuls on a memset tile keep the PE busy
        # through the initial DMA wait so HAM un-throttles (K=8/8) before the
        # first real matmul; otherwise the first ~7us of real work runs at
        # half clock. Results go to a scratch psum tile nothing reads.
        warm_t = const.tile([128, 512], BF16, tag="warm")
        nc.vector.memset(warm_t[:], 0.0)
        for wi in range(26):
            wps = p_pv.tile([64, T], F32, tag="pv", name="pv")
            nc.tensor.matmul(
                wps[:],
                lhsT=warm_t[:, 0:64],
                rhs=warm_t[:, 256 : 256 + T],
                start=True,
                stop=True,
            )

        # ---- main loop over S pieces (Q-proj slotted inside piece 0 so the
        # PE starts on K-proj as soon as the first wkv/ctx chunks land) ------
        for pi, (p0, w) in enumerate(pieces):
            scs = [p0 // 128 + j for j in range(w // 128)]
            # K projection: c-outer so each (wkv_c, ctx_c) chunk is consumed
            # as it arrives (keeps the PE from idling >HAM window at startup);
            # both kc-pair psum tiles stay live through the c loop.
            ps_h = [
                p_pe.tile([128, 1024], F32, tag="pe", name="pe") for _ in range(2)
            ]
            for g in range(3):
                for half in range(2):
                    for kci in range(2):
                        kc = 2 * half + kci
                        nc.tensor.matmul(
                            ps_h[half][:, kci * w : (kci + 1) * w],
                            lhsT=wkv8_v[:, 2 * g : 2 * g + 2, kc * 128 : (kc + 1) * 128],
                            rhs=ctx8_sb[:, 2 * g : 2 * g + 2, p0 : p0 + w],
                            start=(g == 0),
                            stop=(g == 2),
                            perf_mode=mybir.MatmulPerfMode.DoubleRow,
                        )
            kt_v = kt_t[:].rearrange("p (kc s) -> p kc s", s=s_pad)
            for half in range(2):
                nc.vector.tensor_copy(
                    kt_v[:, 2 * half : 2 * half + 2, p0 : p0 + w],
                    ps_h[half][:, 0 : 2 * w].rearrange("p (kc s) -> p kc s", s=w),
                )
            if pi == 0:
                # Q projection: x/wq arrive on the scalar queue while the
                # K-proj trickle runs; this also plugs a PE gap before V-proj
                ps_q = p_pe.tile([128, 1024], F32, tag="pe", name="pe")
                for qc in range(4):
                    for ec in range(4):
                        nc.tensor.matmul(
                            ps_q[:, qc * T : qc * T + T],
                            lhsT=wq_t[:, ec * 512 + qc * 128 : ec * 512 + (qc + 1) * 128],
                            rhs=x_t[:, ec * T : (ec + 1) * T],
                            start=(ec == 0),
                            stop=(ec == 3),
                        )
                nc.vector.tensor_copy(qt_t[:], ps_q[:])
            # V' projection: sc pairs share one psum tile
            for g in range(0, len(scs), 2):
                pair = scs[g : g + 2]
                ps = p_pe.tile([128, 1024], F32, tag="pe", name="pe")
                for j, sc in enumerate(pair):
                    for c in range(6):
                        nc.tensor.matmul(
                            ps[:, j * 512 : (j + 1) * 512],
                            lhsT=ctx_t[:, c * s_pad + sc * 128 : c * s_pad + (sc + 1) * 128],
                            rhs=wkv_t[:, c * 512 : (c + 1) * 512],
                            start=(c == 0),
                            stop=(c == 5),
                        )
                nc.vector.tensor_copy(
                    vp_v[:, pair[0] : pair[0] + len(pair), :, 0:64],
                    ps[:, 0 : len(pair) * 512].rearrange(
                        "p (sc h d) -> p sc h d", h=8, d=64
                    ),
                )
            nc.vector.tensor_copy(
                vp_v[:, scs[0] : scs[0] + len(scs), :, 64:65],
                m01h_t[:, scs[0] * 8 : (scs[0] + len(scs)) * 8].rearrange(
                    "p (sc h o) -> p sc h o", h=8, o=1
                ),
            )
            # scores + exp + PV, per head pair
            for kc in range(4):
                pe0 = p_pe.tile([128, 1024], F32, tag="pe", name="pe")
                pe1 = p_pe.tile([128, 1024], F32, tag="pe", name="pe")
                for j, sc in enumerate(scs):
                    nc.tensor.matmul(
                        pe0[:, j * T : (j + 1) * T],
                        lhsT=kt_t[0:64, kc * s_pad + sc * 128 : kc * s_pad + (sc + 1) * 128],
                        rhs=qt_t[0:64, kc * T : (kc + 1) * T],
                        start=True,
                        stop=True,
                        tile_position=(0, 0),
                    )
                    nc.tensor.matmul(
                        pe1[:, j * T : (j + 1) * T],
                        lhsT=kt_t[64:128, kc * s_pad + sc * 128 : kc * s_pad + (sc + 1) * 128],
                        rhs=qt_t[64:128, kc * T : (kc + 1) * T],
                        start=True,
                        stop=True,
                        tile_position=(64, 0),
                    )
                e0 = work.tile([128, 1024], BF16, tag="exp", bufs=6, name="exp")
                nc.scalar.activation(
                    e0[:, 0 : w * 2], pe0[:, 0 : w * 2], mybir.ActivationFunctionType.Exp
                )
                e1 = work.tile([128, 1024], BF16, tag="exp", bufs=6, name="exp")
                nc.scalar.activation(
                    e1[:, 0 : w * 2], pe1[:, 0 : w * 2], mybir.ActivationFunctionType.Exp
                )
                pv0 = p_pv.tile([65, T], F32, tag="pv", name="pv")
                pv1 = p_pv.tile([65, T], F32, tag="pv", name="pv")
                for j, sc in enumerate(scs):
                    o0 = (sc * 8 + 2 * kc) * 65
                    o1 = (sc * 8 + 2 * kc + 1) * 65
                    nc.tensor.matmul(
                        pv0[:],
                        lhsT=vp_t[:, o0 : o0 + 65],
                        rhs=e0[:, j * T : (j + 1) * T],
                        start=(j == 0),
                        stop=(j == len(scs) - 1),
                    )
                    nc.tensor.matmul(
                        pv1[:],
                        lhsT=vp_t[:, o1 : o1 + 65],
                        rhs=e1[:, j * T : (j + 1) * T],
                        start=(j == 0),
                        stop=(j == len(scs) - 1),
                    )
                if pi == 0:
                    nc.vector.tensor_copy(pvacc_t[2 * kc][:], pv0[:])
                    nc.vector.tensor_copy(pvacc_t[2 * kc + 1][:], pv1[:])
                else:
                    nc.vector.tensor_add(pvacc_t[2 * kc][:], pvacc_t[2 * kc][:], pv0[:])
                    nc.vector.tensor_add(
                        pvacc_t[2 * kc + 1][:], pvacc_t[2 * kc + 1][:], pv1[:]
                    )
                if pi == len(pieces) - 1:
                    if kc >= 1:
                        emit_norm(kc - 1)
                    if kc == 3:
                        emit_norm(2)
                        emit_norm(3)

        # ---- out projection: per-head K=64 contraction against the
        # head-major Wo layout; no partition shifts so each head's OT feeds
        # matmuls straight from the DVE mul. 4 accumulators in 4 banks.
        ps_outA = p_pe.tile([128, 1024], F32, tag="pe", name="pe")
        ps_outB = p_pe.tile([128, 1024], F32, tag="pe", name="pe")
        out_slot = [(ps_outA, 0), (ps_outA, 512), (ps_outB, 0), (ps_outB, 512)]
        for h in range(8):
            for eo in range(4):
                pt, off = out_slot[eo]
                nc.tensor.matmul(
                    pt[:, off : off + T],
                    lhsT=woh_t[:, h * 512 + eo * 128 : h * 512 + (eo + 1) * 128],
                    rhs=oth_t[h][:],
                    start=(h == 0),
                    stop=(h == 7),
                )
        for eo in range(4):
            pt, off = out_slot[eo]
            nc.vector.tensor_scalar_add(
                osb_t[:, eo * T : (eo + 1) * T],
                pt[:, off : off + T],
                bo_t[:, eo : eo + 1],
            )
            eng = nc.sync if eo % 2 == 0 else nc.gpsimd
            eng.dma_start(outT_d[eo], osb_t[:, eo * T : (eo + 1) * T])
        if debug:
            nc.gpsimd.dma_start(dbg_qt, qt_t[:])
            nc.gpsimd.dma_start(dbg_kt, kt_t[:])
            nc.gpsimd.dma_start(dbg_vp, vp_t[:])
            pass
            for h in range(8):
                nc.gpsimd.dma_start(dbg_pv[h], pvacc_t[h][:])
            for cc in range(4):
                nc.gpsimd.dma_start(dbg_ot[cc][0:64], oth_t[2 * cc][:])
                nc.gpsimd.dma_start(dbg_ot[cc][64:128], oth_t[2 * cc + 1][:])

    nc.compile()
    return nc


_NC_CACHE = {}


def _get_nc(s_pad, debug=False):
    key = (s_pad, debug)
    if key not in _NC_CACHE:
        _NC_CACHE[key] = _build_program(s_pad, debug)
    return _NC_CACHE[key]


def _prep_in_maps(x, context, key_padding_mask, Wq, Wkv, Wo, bo):
    keep = ~np.asarray(key_padding_mask)
    kept = keep.sum(axis=1)
    s_pad = max(128, -(-int(kept.max()) // 128) * 128)
    n_sc = s_pad // 128

    wqT = (np.ascontiguousarray(Wq.T) * np.float32(D**-0.5 / 32.0)).astype(NPBF16)
    wkv8 = np.ascontiguousarray(Wkv[:512].T * np.float32(32.0)).astype(
        ml_dtypes.float8_e4m3
    )
    wkvT = np.ascontiguousarray(Wkv[512:].T).astype(NPBF16)
    woh = np.ascontiguousarray(
        Wo.T.reshape(8, 64, 512).transpose(1, 0, 2).reshape(64, 8 * 512)
    ).astype(NPBF16)
    bo_r = np.ascontiguousarray(bo.reshape(4, 128).T).astype(np.float32)
    in_maps = []
    for b in range(B):
        k = int(kept[b])
        ctxT = np.zeros((KV, s_pad), dtype=NPBF16)
        ctxT[:, :k] = context[b][keep[b]].T.astype(NPBF16)
        ctx8 = np.zeros((KV, s_pad), dtype=ml_dtypes.float8_e4m3)
        ctx8[:, :k] = context[b][keep[b]].T.astype(ml_dtypes.float8_e4m3)
        xT = np.ascontiguousarray(x[b].T).astype(NPBF16)
        # ones-column pattern: onesp[p, sc*8 + h] = 1 if sc*128+p < k
        live = (np.arange(s_pad) < k).astype(NPBF16).reshape(n_sc, 128).T
        onesp = np.ascontiguousarray(np.repeat(live, 8, axis=1))
        in_maps.append(
            dict(ctxT=ctxT, ctx8=ctx8, xT=xT, onesp=onesp, wqT=wqT, wkv8=wkv8, wkvT=wkvT, woh=woh, bo_r=bo_r)
        )
    return s_pad, in_maps


def _run(inputs, trace=False, debug=False, **kw):
    s_pad, in_maps = _prep_in_maps(**inputs)
    nc = _get_nc(s_pad, debug)
    res = bass_utils.run_bass_kernel_spmd(
        nc, in_maps, core_ids=list(range(NC_CORES)), trace=trace, **kw
    )
    out = np.stack(
        [res.results[b]["outT"].reshape(E, T).T for b in range(B)]
    ).astype(np.float32)
    return out, res


def kernel(**inputs):
    out, _ = _run(inputs, trace=False)
    return out


if __name__ == "__main__":
    rng = np.random.default_rng(0)
    ins = dict(
        x=rng.standard_normal((B, T, E), dtype=np.float32),
        context=rng.standard_normal((B, S, KV), dtype=np.float32),
        key_padding_mask=rng.integers(0, 2, (B, S)).astype(bool),
        Wq=(rng.standard_normal((512, E), dtype=np.float32) * 0.02),
        Wkv=(rng.standard_normal((1024, KV), dtype=np.float32) * 0.02),
        Wo=(rng.standard_normal((E, 512), dtype=np.float32) * 0.02),
        bo=np.zeros(E, dtype=np.float32),
    )
    out = kernel(**ins)
    print("out", out.shape, out.dtype, np.abs(out).mean())
